# revision 1
# baseline (speedup 1.0000x reference)
"""Trainium2 Bass kernel for windowed (sparse) cross-attention.

Computation (per batch row b of x:(B=2048, N=64, D=512)):
  q/k/v = x @ Wq/Wk/Wv, split into 8 heads of dim 64.
  13 temporal windows of size 16, stride 4 over N=64; softmax attention within
  each window; overlapping window outputs are accumulated and divided by the
  per-position window count; out = value @ Wout + bout.

Strategy (pure data parallel over 8 NeuronCores, batch-sharded):
  - Host pre-transposes the x shard to xT (D, T) and casts operands to bf16.
  - Projections produce qT/kT (inner-on-partitions) and v (tokens-on-partitions).
  - Scores S'[m,n] = k_m . q_n are computed per (2-batch x 2-head) "quad" with
    K=64 matmuls using tile_position row halves; the full 64x64 score block per
    pair is materialized (windows are diagonal 16x16 sub-blocks of it).
  - Window softmax is linear-algebra-ified: with E = exp(S*scale),
      window sums   s[w, n] = (MaskStack^T @ E)        (one matmul)
      R'[m, n] = MaskStack @ (U * 1/s)                 (one matmul)
      P'[m, n] = E * R'                                (elementwise)
    where U[w,n] = 1[n in window w] / cnt[n].  Then value^T = v^T-contracted
    AV matmuls over P' columns.  This makes the entire softmax+window-overlap
    accumulation 2 small matmuls + 3 vector ops per 2-quad unit.
  - Output projection consumes value^T chunks as stationary operands and
    produces the output in natural (token, D) layout; bias added on DVE.
"""

import sys

if "/opt/trn_rl_repo" not in sys.path:
    sys.path.insert(0, "/opt/trn_rl_repo")

import numpy as np
import ml_dtypes

import concourse.bass as bass
import concourse.tile as tile
from concourse import mybir
from concourse.bass_utils import run_bass_kernel_spmd

BF16 = mybir.dt.bfloat16
F32 = mybir.dt.float32
NP_BF16 = ml_dtypes.bfloat16

# Problem constants (hardcoded per contract)
B, N, D = 2048, 64, 512
NCORES = 8
BC = B // NCORES          # batch rows per core
T_FULL = BC * N           # tokens per core = 16384
HEADS, DH = 8, 64
WINDOW, STRIDE, NW = 16, 4, 13
SCALE = DH ** -0.5
TB = 512                  # tokens per block (8 batch rows)

# stash for test harness introspection
last_results = None


def _split_waits(nc, keep=1):
    """walrus in this toolchain supports only one embedded sync wait per
    instruction; hoist excess waits onto standalone EventSemaphore
    instructions on the same engine queue (FIFO => executes first)."""
    ctr = 0
    for f in nc.m.functions:
        for blk in f.blocks:
            il = blk.instructions
            out = []
            changed = False
            for inst in il:
                si = inst.sync_info
                if si is not None and len(si.on_wait) > keep:
                    waits = list(si.on_wait)
                    SyncInfo = type(si)
                    for w in waits[:-keep]:
                        evs = mybir.InstEventSemaphore(
                            name=f"WSPLIT-{ctr}", ins=[], outs=[]
                        )
                        ctr += 1
                        evs.engine = inst.engine
                        evs.sync_info = SyncInfo(on_wait=[w], on_update=[])
                        out.append(evs)
                    inst.sync_info = SyncInfo(
                        on_wait=waits[-keep:], on_update=list(si.on_update)
                    )
                    changed = True
                out.append(inst)
            if changed:
                il[:] = out
    return ctr


def _window_consts():
    idx = np.arange(NW)[:, None] * STRIDE + np.arange(WINDOW)[None, :]
    cnt = np.zeros(N, dtype=np.float64)
    np.add.at(cnt, idx, 1.0)
    member = np.zeros((N, NW), dtype=np.float64)  # member[m, w] = m in window w
    for w in range(NW):
        member[idx[w], w] = 1.0
    mask_s = np.zeros((128, 26), dtype=np.float64)
    mask_s[:64, :13] = member
    mask_s[64:, 13:] = member
    mask_t = mask_s.T.copy()
    u = np.zeros((26, 512), dtype=np.float64)
    for j in range(512):
        s = ((j % 256) // 64) % 2
        n = j % 64
        u[s * 13:(s + 1) * 13, j] = member[n] / cnt[n]
    return (
        mask_s.astype(NP_BF16),
        mask_t.astype(NP_BF16),
        u.astype(np.float32),
    )


def build_program(T=T_FULL):
    nc = bass.Bass()
    xt_d = nc.dram_tensor("xt", [D, T], BF16, kind="ExternalInput")
    wq_d = nc.dram_tensor("wq", [128, 4, D], BF16, kind="ExternalInput")
    wk_d = nc.dram_tensor("wk", [128, 4, D], BF16, kind="ExternalInput")
    wv_d = nc.dram_tensor("wv", [128, 4, D], BF16, kind="ExternalInput")
    wo_d = nc.dram_tensor("wo", [128, 4, D], BF16, kind="ExternalInput")
    bo_d = nc.dram_tensor("bo", [128, D], F32, kind="ExternalInput")
    ms_d = nc.dram_tensor("ms", [128, 26], BF16, kind="ExternalInput")
    mt_d = nc.dram_tensor("mt", [26, 128], BF16, kind="ExternalInput")
    u_d = nc.dram_tensor("u", [26, 512], F32, kind="ExternalInput")
    out_d = nc.dram_tensor("out", [T, D], F32, kind="ExternalOutput")

    NB = T // TB
    EXP = mybir.ActivationFunctionType.Exp

    with tile.TileContext(nc) as tc:
        with (
            tc.tile_pool(name="consts", bufs=1) as consts,
            tc.tile_pool(name="xtp", bufs=8) as xt_pool,
            tc.tile_pool(name="qkp", bufs=16) as qk_pool,
            tc.tile_pool(name="vp", bufs=8) as v_pool,
            tc.tile_pool(name="ep", bufs=4) as e_pool,
            tc.tile_pool(name="rcp", bufs=4) as rc_pool,
            tc.tile_pool(name="pp", bufs=4) as p_pool,
            tc.tile_pool(name="vtp", bufs=8) as vt_pool,
            tc.tile_pool(name="op", bufs=4) as out_pool,
            tc.tile_pool(name="ps_proj", bufs=2, space="PSUM") as ps_proj,
            tc.tile_pool(name="ps_s", bufs=2, space="PSUM") as ps_s,
            tc.tile_pool(name="ps_w", bufs=1, space="PSUM") as ps_w,
            tc.tile_pool(name="ps_r", bufs=1, space="PSUM") as ps_r,
            tc.tile_pool(name="ps_av", bufs=2, space="PSUM") as ps_av,
        ):
            wq_t = consts.tile([128, 4, D], BF16, tag="wq")
            nc.sync.dma_start(wq_t[:], wq_d[:])
            wk_t = consts.tile([128, 4, D], BF16, tag="wk")
            nc.sync.dma_start(wk_t[:], wk_d[:])
            wv_t = consts.tile([128, 4, D], BF16, tag="wv")
            nc.sync.dma_start(wv_t[:], wv_d[:])
            wo_t = consts.tile([128, 4, D], BF16, tag="wo")
            nc.sync.dma_start(wo_t[:], wo_d[:])
            bo_t = consts.tile([128, D], F32, tag="bo")
            nc.sync.dma_start(bo_t[:], bo_d[:])
            ms_t = consts.tile([128, 26], BF16, tag="ms")
            nc.sync.dma_start(ms_t[:], ms_d[:])
            mt_t = consts.tile([26, 128], BF16, tag="mt")
            nc.sync.dma_start(mt_t[:], mt_d[:])
            u_t = consts.tile([26, 512], F32, tag="u")
            nc.sync.dma_start(u_t[:], u_d[:])

            for blk in range(NB):
                t0 = blk * TB

                # ---- load xT tiles (D on partitions, 4 chunks) ----
                xts = []
                for kc in range(4):
                    xt_t = xt_pool.tile([128, TB], BF16, tag="xt")
                    nc.sync.dma_start(
                        xt_t[:], xt_d[kc * 128:(kc + 1) * 128, t0:t0 + TB]
                    )
                    xts.append(xt_t)

                # ---- qT / kT projections, stored as per-head-half tiles
                # [64, TB] at base partition 0 (avoids partition-offset
                # matmul operands, which wedge this hardware) ----
                qts, kts = [], []
                for wt, lst in ((wq_t, qts), (wk_t, kts)):
                    for c in range(4):
                        ps = ps_proj.tile([128, TB], F32, tag="pp")
                        for kc in range(4):
                            nc.tensor.matmul(
                                ps[:],
                                wt[:, kc, c * 128:(c + 1) * 128],
                                xts[kc][:],
                                start=(kc == 0),
                                stop=(kc == 3),
                            )
                        halves = []
                        for hh in range(2):
                            sb = qk_pool.tile([64, TB], BF16, tag="qk")
                            nc.scalar.copy(sb[:], ps[hh * 64:(hh + 1) * 64, :])
                            halves.append(sb)
                        lst.append(halves)

                # ---- v projection: natural layout [128 tokens, 512 i] ----
                vts = []
                for tt in range(4):
                    ps = ps_proj.tile([128, 512], F32, tag="pp")
                    for kc in range(4):
                        nc.tensor.matmul(
                            ps[:],
                            xts[kc][:, tt * 128:(tt + 1) * 128],
                            wv_t[:, kc, :],
                            start=(kc == 0),
                            stop=(kc == 3),
                        )
                    sb = v_pool.tile([128, 512], BF16, tag="vv")
                    nc.vector.tensor_copy(sb[:], ps[:])
                    vts.append(sb)

                # ---- attention per chunk (2 heads) ----
                vt_out = []
                for c in range(4):
                    qc, kc_t = qts[c], kts[c]
                    av = ps_av.tile([128, 512], F32, tag="av")
                    for tb2 in range(2):
                        # unit: 2 quads (each quad = 2 batch rows x 2 heads)
                        sp = ps_s.tile([128, 512], F32, tag="sp")
                        for qd in range(2):
                            tb = tb2 * 2 + qd
                            for hh in range(2):
                                tcols = slice(tb * 128, (tb + 1) * 128)
                                o = sp[:, qd * 256 + hh * 128:
                                       qd * 256 + (hh + 1) * 128]
                                nc.tensor.matmul(
                                    o, kc_t[hh][:, tcols], qc[hh][:, tcols],
                                    start=True, stop=True,
                                )
                        eu = e_pool.tile([128, 512], BF16, tag="eu")
                        nc.scalar.activation(eu[:], sp[:], EXP, scale=float(SCALE))
                        # window sums for all 4 pairs: [26, 512]
                        sw = ps_w.tile([128, 512], F32, tag="sw")
                        nc.tensor.matmul(sw[:26, :], ms_t[:], eu[:], start=True, stop=True)
                        rc = rc_pool.tile([26, 512], F32, tag="rc")
                        nc.vector.reciprocal(rc[:], sw[:26, :])
                        rcu = rc_pool.tile([26, 512], BF16, tag="rcu")
                        nc.vector.tensor_mul(rcu[:], rc[:], u_t[:])
                        rp = ps_r.tile([128, 512], F32, tag="rp")
                        nc.tensor.matmul(rp[:], mt_t[:], rcu[:], start=True, stop=True)
                        pu = p_pool.tile([128, 512], BF16, tag="pu")
                        nc.vector.tensor_mul(pu[:], eu[:], rp[:])
                        # AV: value^T quad blocks -> av[:, tb*128 + ...]
                        for qd in range(2):
                            tb = tb2 * 2 + qd
                            for hh in range(2):
                                lhsT = vts[tb][
                                    :, c * 128 + hh * 64: c * 128 + hh * 64 + 64
                                ]
                                rhs = pu[:, qd * 256 + hh * 128:
                                         qd * 256 + (hh + 1) * 128]
                                o = av[hh * 64:(hh + 1) * 64,
                                       tb * 128:(tb + 1) * 128]
                                nc.tensor.matmul(o, lhsT, rhs, start=True, stop=True)
                    vt = vt_pool.tile([128, 512], BF16, tag="vt")
                    nc.scalar.copy(vt[:], av[:])
                    vt_out.append(vt)

                # ---- output projection + bias ----
                for tt in range(4):
                    ps = ps_proj.tile([128, 512], F32, tag="pp")
                    for c in range(4):
                        nc.tensor.matmul(
                            ps[:],
                            vt_out[c][:, tt * 128:(tt + 1) * 128],
                            wo_t[:, c, :],
                            start=(c == 0),
                            stop=(c == 3),
                        )
                    ob = out_pool.tile([128, 512], F32, tag="ob")
                    nc.vector.tensor_add(ob[:], ps[:], bo_t[:])
                    nc.sync.dma_start(
                        out_d[t0 + tt * 128: t0 + (tt + 1) * 128, :], ob[:]
                    )
    return nc


def _prep_shared(Wq, Wk, Wv, Wout, bout):
    def warr(w):
        return np.ascontiguousarray(
            w.astype(np.float32).reshape(4, 128, D).transpose(1, 0, 2)
        ).astype(NP_BF16)

    mask_s, mask_t, u = _window_consts()
    return {
        "wq": warr(Wq),
        "wk": warr(Wk),
        "wv": warr(Wv),
        "wo": warr(Wout),
        "bo": np.ascontiguousarray(
            np.broadcast_to(bout.astype(np.float32), (128, D))
        ),
        "ms": mask_s,
        "mt": mask_t,
        "u": u,
    }


def kernel(x, Wq, Wk, Wv, Wout, bout):
    global last_results
    x = np.asarray(x, dtype=np.float32)
    shared = _prep_shared(
        np.asarray(Wq), np.asarray(Wk), np.asarray(Wv),
        np.asarray(Wout), np.asarray(bout),
    )
    in_maps = []
    for ci in range(NCORES):
        xs = x[ci * BC:(ci + 1) * BC].reshape(T_FULL, D)
        xt = np.ascontiguousarray(xs.T).astype(NP_BF16)
        in_maps.append({"xt": xt, **shared})

    nc = build_program(T_FULL)
    _split_waits(nc)
    res = run_bass_kernel_spmd(nc, in_maps, list(range(NCORES)))
    last_results = res
    outs = [
        res.results[ci]["out"].astype(np.float32).reshape(BC, N, D)
        for ci in range(NCORES)
    ]
    return np.concatenate(outs, axis=0)



# revision 10
# speedup vs baseline: 2.7642x; 2.7642x over previous
"""Trainium2 Bass kernel for windowed (sparse) cross-attention.

Computation (per batch row b of x:(B=2048, N=64, D=512)):
  q/k/v = x @ Wq/Wk/Wv, split into 8 heads of dim 64.
  13 temporal windows of size 16, stride 4 over N=64; softmax attention within
  each window; overlapping window outputs are accumulated and divided by the
  per-position window count; out = value @ Wout + bout.

Strategy (pure data parallel over 8 NeuronCores, batch-sharded), v2:
  - Q/K projections run in fp8e4 DoubleRow mode (0.5 cycles/row): weights are
    prescaled by 64 (keeps them out of the fp8 denormal range), x is cast to
    fp8; the 64*64 score scale folds into the softmax exp scale.  V and the
    output projection stay bf16 for accuracy.
  - Scores are computed per (2-batch-row x 2-head) quad into [128, 512] PSUM
    tiles; the head-1 operands are read at partition offset 64 directly
    (tile_position=(64,0)), so q/k need only one [128,512] PSUM->SBUF copy
    per chunk.
  - Window softmax via mask matmuls: E = exp(S*scale); window sums for FOUR
    units are packed into ONE PSUM bank with quadrant column-offset writes
    (tile_position=(0,32u)).  1/s is computed as exp(-ln s) on the ACT engine
    (ln+exp share one activation table; DVE reciprocal is 5x slower).
    R' = MaskBD @ (u * 1/s) uses block-diagonal mask constants so the rcu
    operand is read at partition 0 (partition-offset matmul operands wedge
    this hardware).  P = E * R'; AV matmuls then produce value^T.
  - The tensor engine stream interleaves block i's attention with block
    (i+1)'s projections so the PE never idles (it only reaches its 2.4 GHz
    p-state after ~3us of continuous execution).
  - Engine split: ACT = exp/ln/qk copies, DVE = pu/bias/v/vt copies,
    Pool = rcu multiplies (GPSIMD cannot access PSUM).
"""

import sys

if "/opt/trn_rl_repo" not in sys.path:
    sys.path.insert(0, "/opt/trn_rl_repo")

import numpy as np
import ml_dtypes

import concourse.bass as bass
import concourse.tile as tile
from concourse import mybir
from concourse.bass_utils import run_bass_kernel_spmd

BF16 = mybir.dt.bfloat16
F32 = mybir.dt.float32
F8 = mybir.dt.float8e4
NP_BF16 = ml_dtypes.bfloat16
NP_F8 = ml_dtypes.float8_e4m3fn

# Problem constants (hardcoded per contract)
B, N, D = 2048, 64, 512
NCORES = 8
BC = B // NCORES          # batch rows per core
T_FULL = BC * N           # tokens per core = 16384
HEADS, DH = 8, 64
WINDOW, STRIDE, NW = 16, 4, 13
SCALE = DH ** -0.5
WSCALE = 64.0             # fp8 weight prescale
EXP_SCALE = SCALE / (WSCALE * WSCALE)
TB = 512                  # tokens per block (8 batch rows)

EXP = mybir.ActivationFunctionType.Exp
LN = mybir.ActivationFunctionType.Ln
DR = mybir.MatmulPerfMode.DoubleRow

import os as _os
ABL_BF16QK = bool(_os.environ.get("ABL_BF16QK"))
ABL_HALF = bool(_os.environ.get("ABL_HALF"))
ABL_NOQUAD = bool(_os.environ.get("ABL_NOQUAD"))
ABL_DVRECIP = bool(_os.environ.get("ABL_DVRECIP"))

# stash for test harness introspection
last_results = None


def _split_waits(nc, keep=1):
    """walrus in this toolchain supports only one embedded sync wait per
    instruction; hoist excess waits onto standalone EventSemaphore
    instructions on the same engine queue (FIFO => executes first)."""
    ctr = 0
    for f in nc.m.functions:
        for blk in f.blocks:
            il = blk.instructions
            out = []
            changed = False
            for inst in il:
                si = inst.sync_info
                if si is not None and len(si.on_wait) > keep:
                    waits = list(si.on_wait)
                    SyncInfo = type(si)
                    for w in waits[:-keep]:
                        evs = mybir.InstEventSemaphore(
                            name=f"WSPLIT-{ctr}", ins=[], outs=[]
                        )
                        ctr += 1
                        evs.engine = inst.engine
                        evs.sync_info = SyncInfo(on_wait=[w], on_update=[])
                        out.append(evs)
                    inst.sync_info = SyncInfo(
                        on_wait=waits[-keep:], on_update=list(si.on_update)
                    )
                    changed = True
                out.append(inst)
            if changed:
                il[:] = out
    return ctr


def _window_consts():
    idx = np.arange(NW)[:, None] * STRIDE + np.arange(WINDOW)[None, :]
    cnt = np.zeros(N, dtype=np.float64)
    np.add.at(cnt, idx, 1.0)
    member = np.zeros((N, NW), dtype=np.float64)  # member[m, w] = m in window w
    for w in range(NW):
        member[idx[w], w] = 1.0
    # mask_s[m, w] over a 128-token slab (2 batch rows stacked)
    mask_s = np.zeros((128, 26), dtype=np.float64)
    mask_s[:64, :13] = member
    mask_s[64:, 13:] = member
    # ms32: [128, 32]; pad windows 26..31 get a single 1 so their window sum
    # is positive (keeps ln finite); their u4/mtbd rows are zero.
    ms32 = np.zeros((128, 32), dtype=np.float64)
    ms32[:, :26] = mask_s
    ms32[0, 26:] = 1.0
    # mtbd: [128, 4, 128]; block-diagonal mask_s^T replicas so rp matmuls
    # contract over the full 128 partitions of the batched rcu tile.
    mtbd = np.zeros((128, 4, 128), dtype=np.float64)
    for u in range(4):
        mtbd[32 * u:32 * u + 26, u, :] = mask_s.T
    # u4: [128, 512]; u[w, j] replicated into each 32-row quadrant.
    u = np.zeros((26, 512), dtype=np.float64)
    for j in range(512):
        s = (j % 128) // 64
        n = j % 64
        u[s * 13:(s + 1) * 13, j] = member[n] / cnt[n]
    u4 = np.zeros((128, 512), dtype=np.float64)
    for q in range(4):
        u4[32 * q:32 * q + 26] = u
    return (
        ms32.astype(NP_BF16),
        np.ascontiguousarray(mtbd).astype(NP_BF16),
        u4.astype(np.float32),
    )


def build_program(T=T_FULL):
    nc = bass.Bass()
    xt_d = nc.dram_tensor("xt", [D, T], BF16, kind="ExternalInput")
    x8_d = nc.dram_tensor("x8", [128, 2, 2, T], F8, kind="ExternalInput")
    wq8_d = nc.dram_tensor("wq8", [128, 2, 4, 2, 128], F8, kind="ExternalInput")
    wk8_d = nc.dram_tensor("wk8", [128, 2, 4, 2, 128], F8, kind="ExternalInput")
    wv_d = nc.dram_tensor("wv", [128, 4, D], BF16, kind="ExternalInput")
    wo_d = nc.dram_tensor("wo", [128, 4, D], BF16, kind="ExternalInput")
    bo_d = nc.dram_tensor("bo", [128, D], F32, kind="ExternalInput")
    ms32_d = nc.dram_tensor("ms32", [128, 32], BF16, kind="ExternalInput")
    mtbd_d = nc.dram_tensor("mtbd", [128, 4, 128], BF16, kind="ExternalInput")
    u4_d = nc.dram_tensor("u4", [128, 512], F32, kind="ExternalInput")
    wq_d = nc.dram_tensor("wq", [128, 4, D], BF16, kind="ExternalInput")
    wk_d = nc.dram_tensor("wk", [128, 4, D], BF16, kind="ExternalInput")
    mt26_d = nc.dram_tensor("mt26", [26, 128], BF16, kind="ExternalInput")
    out_d = nc.dram_tensor("out", [T, D], F32, kind="ExternalOutput")

    NB = T // TB

    with tile.TileContext(nc) as tc:
        with (
            tc.tile_pool(name="consts", bufs=1) as consts,
            tc.tile_pool(name="xtp", bufs=2) as xt_pool,
            tc.tile_pool(name="qkp", bufs=2) as qk_pool,
            tc.tile_pool(name="vp", bufs=2) as v_pool,
            tc.tile_pool(name="eup", bufs=2) as e_pool,
            tc.tile_pool(name="rcp", bufs=2) as rc_pool,
            tc.tile_pool(name="pup", bufs=2) as p_pool,
            tc.tile_pool(name="vtp", bufs=2) as vt_pool,
            tc.tile_pool(name="op", bufs=2) as out_pool,
            tc.tile_pool(name="ps_po", bufs=3, space="PSUM") as ps_po,
            tc.tile_pool(name="ps_s", bufs=2, space="PSUM") as ps_s,
            tc.tile_pool(name="ps_w", bufs=1, space="PSUM") as ps_w,
            tc.tile_pool(name="ps_r", bufs=2, space="PSUM") as ps_r,
        ):
            wq8_t = consts.tile([128, 2, 4, 2, 128], F8, tag="wq8")
            nc.sync.dma_start(wq8_t[:], wq8_d[:])
            wk8_t = consts.tile([128, 2, 4, 2, 128], F8, tag="wk8")
            nc.sync.dma_start(wk8_t[:], wk8_d[:])
            wv_t = consts.tile([128, 4, D], BF16, tag="wv")
            nc.sync.dma_start(wv_t[:], wv_d[:])
            wo_t = consts.tile([128, 4, D], BF16, tag="wo")
            nc.sync.dma_start(wo_t[:], wo_d[:])
            bo_t = consts.tile([128, D], F32, tag="bo")
            nc.sync.dma_start(bo_t[:], bo_d[:])
            ms32_t = consts.tile([128, 32], BF16, tag="ms32")
            nc.sync.dma_start(ms32_t[:], ms32_d[:])
            mtbd_t = consts.tile([128, 4, 128], BF16, tag="mtbd")
            nc.sync.dma_start(mtbd_t[:], mtbd_d[:])
            u4_t = consts.tile([128, 512], F32, tag="u4")
            nc.sync.dma_start(u4_t[:], u4_d[:])
            wq_t = consts.tile([128, 4, D], BF16, tag="wq")
            nc.sync.dma_start(wq_t[:], wq_d[:])
            wk_t = consts.tile([128, 4, D], BF16, tag="wk")
            nc.sync.dma_start(wk_t[:], wk_d[:])
            mt26_t = consts.tile([26, 128], BF16, tag="mt26")
            nc.sync.dma_start(mt26_t[:], mt26_d[:])

            # per-block tile state
            S = {}

            def dma_in(i):
                if i >= NB:
                    return
                t0 = i * TB
                st = S.setdefault(i, {})
                st["xt"] = []
                for kc in range(4):
                    xt_t = xt_pool.tile([128, TB], BF16, tag=f"xt{kc}")
                    nc.sync.dma_start(
                        xt_t[:], xt_d[kc * 128:(kc + 1) * 128, t0:t0 + TB]
                    )
                    st["xt"].append(xt_t)
                st["x8"] = []
                for g in range(2):
                    x8_t = xt_pool.tile([128, 2, TB], F8, tag=f"x8{g}")
                    nc.sync.dma_start(x8_t[:], x8_d[:, g, :, t0:t0 + TB])
                    st["x8"].append(x8_t)

            def emit_qkproj(i, c):
                """fp8 DR projections for chunk c: q and k."""
                if i >= NB:
                    return
                st = S[i]
                qs = st.setdefault("q", {})
                ks = st.setdefault("k", {})
                for name, wt, wtb, dst in (
                    ("q", wq8_t, wq_t, qs), ("k", wk8_t, wk_t, ks)
                ):
                    ps = ps_po.tile([128, TB], F32, tag="po")
                    if ABL_BF16QK:
                        for kc in range(4):
                            nc.tensor.matmul(
                                ps[:],
                                wtb[:, kc, c * 128:(c + 1) * 128],
                                st["xt"][kc][:],
                                start=(kc == 0), stop=(kc == 3),
                            )
                    else:
                        for g in range(2):
                            nc.tensor.matmul(
                                ps[:],
                                wt[:, g, c, :, :],
                                st["x8"][g][:],
                                start=(g == 0), stop=(g == 1),
                                perf_mode=DR,
                            )
                    sb = qk_pool.tile([128, TB], BF16, tag=f"{name}{c}")
                    nc.scalar.copy(sb[:], ps[:])
                    dst[c] = sb
                    # partition-offset matmul operands wedge this hardware, so
                    # stage head-1 rows at partition 0 via SBUF->SBUF DMA
                    # (DMA engines are nearly idle; ACT is the scarce engine).
                    h1 = qk_pool.tile([64, TB], BF16, tag=f"{name}h{c}")
                    nc.sync.dma_start(h1[:], sb[64:128, :])
                    st.setdefault(name + "h", {})[c] = h1

            def emit_vproj(i, tt):
                if i >= NB:
                    return
                st = S[i]
                ps = ps_po.tile([128, 512], F32, tag="po")
                for kc in range(4):
                    nc.tensor.matmul(
                        ps[:],
                        st["xt"][kc][:, tt * 128:(tt + 1) * 128],
                        wv_t[:, kc, :],
                        start=(kc == 0), stop=(kc == 3),
                    )
                sb = v_pool.tile([128, 512], BF16, tag=f"v{tt}")
                nc.vector.tensor_copy(sb[:], ps[:])
                st.setdefault("v", {})[tt] = sb

            def emit_scores(i, u):
                """unit u = (c, tb2): scores for 2 quads + exp."""
                st = S[i]
                c, tb2 = u // 2, u % 2
                qc, kc = st["q"][c], st["k"][c]
                sp = ps_s.tile([128, 512], F32, tag="sp")
                for qd in range(2):
                    tb = tb2 * 2 + qd
                    tcols = slice(tb * 128, (tb + 1) * 128)
                    for hh in range(2):
                        o = sp[:, qd * 256 + hh * 128:qd * 256 + (hh + 1) * 128]
                        if hh == 1:
                            lhs = st["kh"][c][:, tcols]
                            rhs = st["qh"][c][:, tcols]
                        else:
                            lhs = kc[0:64, tcols]
                            rhs = qc[0:64, tcols]
                        nc.tensor.matmul(o, lhs, rhs, start=True, stop=True)
                eu = e_pool.tile([128, 512], BF16, tag=f"eu{u}")
                esc = SCALE if ABL_BF16QK else EXP_SCALE
                nc.scalar.activation(eu[:], sp[:], EXP, scale=float(esc))
                st.setdefault("eu", {})[u] = eu

            def emit_sw(i, u):
                """window-sum matmul of unit u into quadrant u%4 of the
                group's shared PSUM bank."""
                st = S[i]
                g, uq = u // 4, u % 4
                sws = st.setdefault("sw", {})
                if ABL_NOQUAD:
                    swt = ps_w.tile([128, 512], F32, tag="sw", name=f"swt{i}_{u}")
                    sws[u] = swt
                    nc.tensor.matmul(
                        swt[0:32, :], ms32_t[:], st["eu"][u][:],
                        start=True, stop=True,
                    )
                    return
                if g not in sws:
                    sws[g] = ps_w.tile([128, 512], F32, tag="sw", name=f"swt{i}_{g}")
                swt = sws[g]
                nc.tensor.matmul(
                    swt[32 * uq:32 * (uq + 1), :],
                    ms32_t[:], st["eu"][u][:],
                    start=True, stop=True,
                    tile_position=(0, 32 * uq),
                    skip_group_check=True,
                )

            def emit_lnexp(i, g):
                """rcu_g = u4 * exp(-ln(sw_g)) for a 4-unit group.
                With ABL_NOQUAD, g is a UNIT index on rows 0:32."""
                st = S[i]
                rows = slice(0, 32) if ABL_NOQUAD else slice(0, 128)
                nr = 32 if ABL_NOQUAD else 128
                src_ = st["sw"][g][rows, :]
                rci = rc_pool.tile([nr, 512], F32, tag=f"rci{g % 2}",
                                   name=f"rci{i}_{g}")
                if ABL_DVRECIP:
                    nc.vector.reciprocal(rci[:], src_)
                else:
                    lns = rc_pool.tile([nr, 512], F32, tag=f"lns{g % 2}",
                                       name=f"lns{i}_{g}")
                    nc.scalar.activation(lns[:], src_, LN, scale=1.0)
                    nc.scalar.activation(rci[:], lns[:], EXP, scale=-1.0)
                rcu = rc_pool.tile([nr, 512], BF16, tag=f"rcu{g % 2}",
                                   name=f"rcu{i}_{g}")
                if _os.environ.get("KV2_RCU_DVE") or ABL_NOQUAD:
                    nc.vector.tensor_mul(rcu[:], rci[:], u4_t[0:nr, :])
                else:
                    nc.gpsimd.tensor_mul(rcu[:], rci[:], u4_t[:])
                st.setdefault("rcu", {})[g] = rcu

            def emit_rp_pu(i, u):
                st = S[i]
                g, uq = u // 4, u % 4
                if ABL_NOQUAD:
                    rp = ps_r.tile([128, 512], F32, tag="rp", name=f"rp{i}_{u}")
                    nc.tensor.matmul(
                        rp[:], mt26_t[:], st["rcu"][u][0:26, :],
                        start=True, stop=True,
                    )
                    pu = p_pool.tile([128, 512], BF16, tag=f"pu{u}",
                                     name=f"pu{i}_{u}")
                    nc.vector.tensor_mul(pu[:], st["eu"][u][:], rp[:])
                    st.setdefault("pu", {})[u] = pu
                    return
                rp = ps_r.tile([128, 512], F32, tag="rp")
                nc.tensor.matmul(
                    rp[:], mtbd_t[:, uq, :], st["rcu"][g][:],
                    start=True, stop=True,
                )
                pu = p_pool.tile([128, 512], BF16, tag=f"pu{u}")
                nc.vector.tensor_mul(pu[:], st["eu"][u][:], rp[:])
                st.setdefault("pu", {})[u] = pu

            def emit_av(i, c):
                """AV for chunk c (both tb2 units) -> value^T tile + copy."""
                st = S[i]
                av = ps_po.tile([128, 512], F32, tag="po")
                for tb2 in range(2):
                    pu = st["pu"][c * 2 + tb2]
                    for qd in range(2):
                        tb = tb2 * 2 + qd
                        for hh in range(2):
                            lhsT = st["v"][tb][
                                :, c * 128 + hh * 64:c * 128 + hh * 64 + 64
                            ]
                            rhs = pu[:, qd * 256 + hh * 128:
                                     qd * 256 + (hh + 1) * 128]
                            o = av[hh * 64:(hh + 1) * 64,
                                   tb * 128:(tb + 1) * 128]
                            nc.tensor.matmul(o, lhsT, rhs, start=True, stop=True)
                vt = vt_pool.tile([128, 512], BF16, tag=f"vt{c}")
                nc.vector.tensor_copy(vt[:], av[:])
                st.setdefault("vt", {})[c] = vt

            def emit_out(i, tt):
                st = S[i]
                t0 = i * TB
                ps = ps_po.tile([128, 512], F32, tag="po")
                for c in range(4):
                    nc.tensor.matmul(
                        ps[:],
                        st["vt"][c][:, tt * 128:(tt + 1) * 128],
                        wo_t[:, c, :],
                        start=(c == 0), stop=(c == 3),
                    )
                ob = out_pool.tile([128, 512], F32, tag=f"ob{tt}")
                nc.vector.tensor_add(ob[:], ps[:], bo_t[:])
                nc.sync.dma_start(
                    out_d[t0 + tt * 128:t0 + (tt + 1) * 128, :], ob[:]
                )

            # ---- prologue: block 0 loads + projections ----
            dma_in(0)
            for c in range(4):
                emit_qkproj(0, c)
            for tt in range(4):
                emit_vproj(0, tt)

            # ---- steady state: attention(i) interleaved with proj(i+1) ----
            import os as _os
            if _os.environ.get("KV2_SEQ"):
                for i in range(NB):
                    j = i + 1
                    dma_in(j)
                    for u in range(8):
                        emit_scores(i, u)
                    if ABL_NOQUAD:
                        for u in range(8):
                            emit_sw(i, u)
                            emit_lnexp(i, u)
                    else:
                        for u in range(4):
                            emit_sw(i, u)
                        emit_lnexp(i, 0)
                        for u in range(4, 8):
                            emit_sw(i, u)
                        emit_lnexp(i, 1)
                    for u in range(8):
                        emit_rp_pu(i, u)
                    for c in range(4):
                        emit_av(i, c)
                    for tt in range(4):
                        emit_out(i, tt)
                    for c in range(4):
                        emit_qkproj(j, c)
                    for tt in range(4):
                        emit_vproj(j, tt)
                    del S[i]
                return nc
            for i in range(NB):
                j = i + 1
                dma_in(j)
                emit_scores(i, 0)
                emit_scores(i, 1)
                emit_qkproj(j, 0)
                emit_scores(i, 2)
                emit_scores(i, 3)
                emit_qkproj(j, 1)
                for u in range(4):
                    emit_sw(i, u)
                emit_lnexp(i, 0)
                emit_scores(i, 4)
                emit_scores(i, 5)
                emit_qkproj(j, 2)
                emit_scores(i, 6)
                emit_scores(i, 7)
                emit_qkproj(j, 3)
                emit_vproj(j, 0)
                emit_rp_pu(i, 0)
                emit_rp_pu(i, 1)
                emit_vproj(j, 1)
                emit_rp_pu(i, 2)
                emit_rp_pu(i, 3)
                emit_av(i, 0)
                emit_av(i, 1)
                for u in range(4, 8):
                    emit_sw(i, u)
                emit_lnexp(i, 1)
                emit_vproj(j, 2)
                emit_vproj(j, 3)
                emit_rp_pu(i, 4)
                emit_rp_pu(i, 5)
                emit_rp_pu(i, 6)
                emit_rp_pu(i, 7)
                emit_av(i, 2)
                emit_av(i, 3)
                for tt in range(4):
                    emit_out(i, tt)
                del S[i]
    return nc


def _prep_shared(Wq, Wk, Wv, Wout, bout):
    def warr8(w):
        # [p, g, c, i, o128]: w row index d = g*256 + i*128 + p, col = c*128 + o
        a = (w.astype(np.float32) * WSCALE).reshape(2, 2, 128, 4, 128)
        return np.ascontiguousarray(a.transpose(2, 0, 3, 1, 4)).astype(NP_F8)

    def warr(w):
        return np.ascontiguousarray(
            w.astype(np.float32).reshape(4, 128, D).transpose(1, 0, 2)
        ).astype(NP_BF16)

    ms32, mtbd, u4 = _window_consts()
    idx = np.arange(NW)[:, None] * STRIDE + np.arange(WINDOW)[None, :]
    member = np.zeros((N, NW))
    for w in range(NW):
        member[idx[w], w] = 1.0
    mask_s = np.zeros((128, 26))
    mask_s[:64, :13] = member
    mask_s[64:, 13:] = member
    return {
        "wq": warr(Wq),
        "wk": warr(Wk),
        "mt26": np.ascontiguousarray(mask_s.T).astype(NP_BF16),
        "wq8": warr8(Wq),
        "wk8": warr8(Wk),
        "wv": warr(Wv),
        "wo": warr(Wout),
        "bo": np.ascontiguousarray(
            np.broadcast_to(bout.astype(np.float32), (128, D))
        ),
        "ms32": ms32,
        "mtbd": mtbd,
        "u4": u4,
    }


def _prep_x(xs):
    """xs: (T, D) f32 -> xt bf16 [D, T], x8 fp8 [128, 2, 2, T]."""
    xt = np.ascontiguousarray(xs.T)
    x8 = np.ascontiguousarray(
        xt.reshape(2, 2, 128, xs.shape[0]).transpose(2, 0, 1, 3)
    ).astype(NP_F8)
    return xt.astype(NP_BF16), x8


def kernel(x, Wq, Wk, Wv, Wout, bout):
    global last_results
    x = np.asarray(x, dtype=np.float32)
    shared = _prep_shared(
        np.asarray(Wq), np.asarray(Wk), np.asarray(Wv),
        np.asarray(Wout), np.asarray(bout),
    )
    in_maps = []
    for ci in range(NCORES):
        xs = x[ci * BC:(ci + 1) * BC].reshape(T_FULL, D)
        xt, x8 = _prep_x(xs)
        in_maps.append({"xt": xt, "x8": x8, **shared})

    nc = build_program(T_FULL)
    _split_waits(nc)
    res = run_bass_kernel_spmd(nc, in_maps, list(range(NCORES)))
    last_results = res
    outs = [
        res.results[ci]["out"].astype(np.float32).reshape(BC, N, D)
        for ci in range(NCORES)
    ]
    return np.concatenate(outs, axis=0)
